# revision 20
# baseline (speedup 1.0000x reference)
"""Trainium2 Bass kernel for nn_Encoder_Attentioner (sparse_attention).

Mathematical collapse: the reference builds aff = Q @ K^T (8192x8192) but only
consumes per-key-batch block sums, which are linear:
    s[bq, i, bk] = q[bq,i] . sum_j k[bk, j]
Folding the q/k/conv 1x1 layers through that sum, the whole computation per
query batch b reduces to
    z[i] = x_b[i] . w_b,        w_b = (conv_w^T + I) q_w^T v_b
    v_b  = s0 * k_w (conv_w + I) d_b + (bias terms)
    d_b  = column-sum of x over all *other* batches' rows
    out  = sigmoid(((z - zmin)/(zmax - zmin) - 0.65) / 0.15)
(additive constants cancel exactly under the min-max normalization; the
q-bias drops entirely).

Distribution over 8 NeuronCores: core b owns query batch b.
  Phase 1 (SPMD): core b computes colsum(x[b]) on device -> (128, 4) f32.
  Host exchanges the eight 2KB sums (pure data movement).
  Phase 2 (SPMD): core b sums the 7 other batches' colsums on device, runs the
  matvec chain, the z row-dot, min-max norm and sigmoid.

Cost-model-aware layout choices: every matmul keeps the large tensor as the
stationary (Ldweights) operand with a (128, 1) output column; z lives in a
(128, 8) pixel layout so the min/max runs as two free-axis reduces, a PE
transpose, one combined (2,128) reduce and a PE dot; biases join the PSUM
accumulations as K=1 matmuls; the sigmoid activation table is pre-warmed by a
dummy activation; and the weight/xT DMA stream is ordered (weights first on
one engine, xT last) so the chain hides entirely under the transfers and the
stream never bubbles on the DMA engines.

Host-side prep is layout only: dtype casts, transposes, folding the residual
identities into the weight matrices, and elementwise constant scaling of one
weight matrix and the two bias vectors.
"""

import numpy as np
import ml_dtypes

import concourse.bass as bass
import concourse.bacc as bacc
import concourse.mybir as mybir
import concourse.tile as tile
from concourse import masks
from concourse.bass_utils import run_bass_kernel_spmd

B, HW, C = 8, 1024, 512
P = 128
NCH = C // P   # 4 column chunks of 128 channels
NT = HW // P   # 8 pixel tiles of 128
N_CORES = 8
BF16 = mybir.dt.bfloat16
F32 = mybir.dt.float32
ATT_SCALE = float(1.0 / np.sqrt(C))                 # 1/sqrt(512)
BIAS_MULT = float((B - 1) * HW)                     # 7168
SCALE0 = ATT_SCALE / BIAS_MULT
THR_SCALE = 1.0 / 0.15
THR_BIAS = -0.65 / 0.15

_k1 = None
_k2 = None
last_results = {}


def _run_spmd(nc, in_maps, core_ids, attempts=3):
    """run_bass_kernel_spmd with retries for transient device errors."""
    import time

    last_err = None
    for i in range(attempts):
        try:
            return run_bass_kernel_spmd(nc, in_maps, core_ids)
        except Exception as e:  # noqa: BLE001 - transient PJRT/NRT failures
            last_err = e
            time.sleep(2.0 * (i + 1))
    raise last_err


def _new_nc():
    return bacc.Bacc(
        "TRN2",
        target_bir_lowering=False,
        debug=False,
        enable_asserts=False,
        num_devices=N_CORES,
    )


def _build_k1():
    """Per core: xb (1024, 512) bf16 -> xsum (128, 4) f32 column sum.

    Column sums via x-stationary matmuls (free Ldweights, 1-wide moving ones)
    directly into the (128, 4) channel-chunked layout phase 2 consumes.
    """
    nc = _new_nc()
    xb = nc.dram_tensor("xb", [HW, C], BF16, kind="ExternalInput")
    xsum = nc.dram_tensor("xsum", [P, NCH], F32, kind="ExternalOutput")
    with tile.TileContext(nc) as tc:
        with (
            tc.tile_pool(name="sb", bufs=1) as sb,
            tc.tile_pool(name="ps", bufs=1, space=bass.MemorySpace.PSUM) as ps,
        ):
            xr = xb.ap().rearrange("(t p) c -> p t c", p=P)
            xt = sb.tile([P, NT, C], BF16)
            half = NT // 2
            nc.sync.dma_start(xt[:, 0:half, :], xr[:, 0:half, :])
            nc.scalar.dma_start(xt[:, half:NT, :], xr[:, half:NT, :])
            ones = sb.tile([P, 1], BF16)
            nc.gpsimd.memset(ones[:], 1.0)
            acc = ps.tile([P, NCH], F32)
            for cc in range(NCH):
                for t in range(NT):
                    nc.tensor.matmul(
                        acc[:, cc : cc + 1],
                        xt[:, t, cc * P : (cc + 1) * P],
                        ones[:],
                        start=(t == 0),
                        stop=(t == NT - 1),
                    )
            res = sb.tile([P, NCH], F32)
            nc.vector.tensor_copy(res[:], acc[:])
            nc.sync.dma_start(xsum.ap(), res[:])
    nc.compile()
    return nc


def _build_k2():
    """Per core: matvec chain + z row-dot + minmax + sigmoid."""
    nc = _new_nc()
    xso = nc.dram_tensor("xso", [B - 1, NCH, P], F32, kind="ExternalInput")
    # [0]: 7168*conv_b; [1]: scale*k_b  (single-partition rows for K=1 matmuls)
    cbt = nc.dram_tensor("cbt", [1, 2, NCH, P], F32, kind="ExternalInput")
    r1 = nc.dram_tensor("r1", [C, C], BF16, kind="ExternalInput")   # (conv_w^T+I)
    kwt = nc.dram_tensor("k_wt", [C, C], BF16, kind="ExternalInput")  # s0*k_w^T
    qw = nc.dram_tensor("q_w", [C, C], BF16, kind="ExternalInput")
    r2 = nc.dram_tensor("r2", [C, C], BF16, kind="ExternalInput")   # conv_w+I
    xT = nc.dram_tensor("xT", [C, HW], BF16, kind="ExternalInput")
    out = nc.dram_tensor("out", [P, NT], F32, kind="ExternalOutput")

    mmin = mybir.AluOpType.min
    mmax = mybir.AluOpType.max
    madd = mybir.AluOpType.add
    mmul = mybir.AluOpType.mult
    mbyp = mybir.AluOpType.bypass
    X = mybir.AxisListType.X

    with tile.TileContext(nc) as tc:
        with (
            tc.tile_pool(name="sb", bufs=1) as sb,
            tc.tile_pool(name="ps", bufs=1, space=bass.MemorySpace.PSUM) as ps,
            tc.tile_pool(name="psmv", bufs=2, space=bass.MemorySpace.PSUM) as psmv,
            tc.tile_pool(name="pse", bufs=1, space=bass.MemorySpace.PSUM) as pse,
        ):
            # --- input DMA stream: smalls, then weights in use order, xT last.
            # Issues spread over SP/Act (HWDGE) and Pool (SWDGE).
            wt = {}

            def load_w(eng, name, h):
                t = sb.tile([P, NCH, C], BF16, tag=name)
                eng.dma_start(t[:], h.ap().rearrange("(t p) c -> p t c", p=P))
                wt[name] = t

            # SP owns the four weights in use order (same-engine pipeline =>
            # deterministic transfer order); smalls lead on Act; xT drains
            # last from Pool/Act. First transfer is big so the stream never
            # bubbles.
            load_w(nc.sync, "r1", r1)
            load_w(nc.sync, "kwt", kwt)
            load_w(nc.sync, "qw", qw)
            load_w(nc.sync, "r2", r2)
            cbt_t = sb.tile([1, 2, NCH, P], F32, tag="cbt")
            nc.scalar.dma_start(cbt_t[:], cbt.ap())
            xso_t = sb.tile([B - 1, NCH, P], F32, tag="xso")
            nc.scalar.dma_start(xso_t[:], xso.ap())

            xT_t = sb.tile([P, NCH, HW], BF16, tag="xT")
            xT_r = xT.ap().rearrange("(t p) i -> p t i", p=P)
            xt_eng = [nc.gpsimd, nc.gpsimd, nc.scalar, nc.scalar]
            for cc in range(NCH):
                xt_eng[cc].dma_start(xT_t[:, cc, :], xT_r[:, cc, :])

            ones7 = sb.tile([B - 1, 1], F32)
            nc.gpsimd.memset(ones7[:], 1.0)
            ones2n = sb.tile([2, 1], F32)
            nc.gpsimd.memset(ones2n[:], -0.15)
            ident = sb.tile([P, P], F32, tag="ident")
            masks.make_identity(nc, ident[:])
            # Warm the Sigmoid activation table off the critical path with a
            # dummy activation in the same bias/scale-by-AP form.
            warm = sb.tile([1, 2], F32, tag="warm")
            nc.gpsimd.memset(warm[:], 0.0)
            nc.scalar.activation(
                warm[:, 0:1],
                warm[:, 0:1],
                mybir.ActivationFunctionType.Sigmoid,
                bias=warm[:, 1:2],
                scale=warm[:, 1:2],
            )

            # d = colsum of the 7 other batches, (128, 4) channel-chunked.
            d_ps = ps.tile([P, NCH], F32, tag="d")
            for cc in range(NCH):
                nc.tensor.matmul(
                    d_ps[:, cc : cc + 1],
                    xso_t[:, cc, :],
                    ones7[:],
                    start=True,
                    stop=True,
                )
            d_bf = sb.tile([P, NCH], BF16, tag="d_bf")
            nc.vector.tensor_copy(d_bf[:], d_ps[:])

            def matvec_T(tiles, vec_bf, out_bf, bias_row=None):
                """out_bf (128,4) bf16 = R^T @ vec (+ bias), R = DRAM matrix.

                The optional bias joins the PSUM accumulation as a K=1 matmul
                against a constant row of xsoc, so the epilogue is a copy.
                """
                mps = psmv.tile([P, NCH], F32, tag="mv")
                for oc in range(NCH):
                    for ic in range(NCH):
                        nc.tensor.matmul(
                            mps[:, oc : oc + 1],
                            tiles[:, ic, oc * P : (oc + 1) * P],
                            vec_bf[:, ic : ic + 1],
                            start=(ic == 0),
                            stop=(ic == NCH - 1 and bias_row is None),
                        )
                    if bias_row is not None:
                        nc.tensor.matmul(
                            mps[:, oc : oc + 1],
                            cbt_t[0:1, bias_row, oc, :],
                            ones7[0:1, :],
                            start=False,
                            stop=True,
                        )
                nc.vector.tensor_copy(out_bf[:], mps[:])

            # XR = (conv_w + I) d + 7168*conv_b
            xr_bf = sb.tile([P, NCH], BF16, tag="xr_bf")
            matvec_T(wt["r1"], d_bf, xr_bf, bias_row=0)
            # v = SCALE0 * (k_w XR) + scale * k_b   (kwt prescaled by SCALE0)
            v_bf = sb.tile([P, NCH], BF16, tag="v_bf")
            matvec_T(wt["kwt"], xr_bf, v_bf, bias_row=1)
            # u = q_w^T v
            u_bf = sb.tile([P, NCH], BF16, tag="u_bf")
            matvec_T(wt["qw"], v_bf, u_bf)
            # w = (conv_w^T + I) u
            w_bf = sb.tile([P, NCH], BF16, tag="w_bf")
            matvec_T(wt["r2"], u_bf, w_bf)

            # z = x_own @ w in (128, 8) pixel layout; each xT channel chunk is
            # consumed as it arrives (cc-outer accumulation into 8 columns).
            zps = ps.tile([P, NT], F32, tag="z")
            for cc in range(NCH):
                for t in range(NT):
                    nc.tensor.matmul(
                        zps[:, t : t + 1],
                        xT_t[:, cc, t * P : (t + 1) * P],
                        w_bf[:, cc : cc + 1],
                        start=(cc == 0),
                        stop=(cc == NCH - 1),
                    )

            # Global min/max: free-axis reduce to (128, [min, -max]), PE
            # transpose to (2, 128), one combined min reduce to (2, 1), then a
            # PE dot with -0.15 gives 0.15*rng and the scalar chain follows.
            mm = sb.tile([P, 2], F32, tag="mm")
            nc.vector.tensor_reduce(mm[:, 0:1], zps[:], axis=X, op=mmin)
            nc.vector.tensor_reduce(mm[:, 1:2], zps[:], axis=X, op=mmax, negate=True)
            mmT = pse.tile([2, P], F32, tag="mmT")
            nc.tensor.transpose(mmT[:], mm[:], ident[:])
            fin = sb.tile([2, 1], F32, tag="fin")
            nc.vector.tensor_reduce(fin[:], mmT[:], axis=X, op=mmin)
            # tt = -0.15 * (mn - mx) = 0.15 * rng
            tt = pse.tile([1, 1], F32, tag="tt")
            nc.tensor.matmul(tt[:], ones2n[:], fin[:], start=True, stop=True)
            scb = sb.tile([1, 2], F32, tag="scb")
            nc.vector.reciprocal_approx_fast(scb[:, 0:1], tt[:])  # scale=T/rng
            t1 = sb.tile([1, 1], F32, tag="t1")
            nc.vector.tensor_tensor(t1[:], fin[0:1, 0:1], scb[:, 0:1], op=mmul)
            nc.vector.tensor_scalar(
                scb[:, 1:2], t1[:], -1.0, THR_BIAS, mmul, madd
            )

            bcs = sb.tile([P, 2], F32, tag="bcs")
            nc.gpsimd.partition_broadcast(bcs[:], scb[:])

            res = sb.tile([P, NT], F32, tag="res")
            nc.scalar.activation(
                res[:],
                zps[:],
                mybir.ActivationFunctionType.Sigmoid,
                bias=bcs[:, 1:2],
                scale=bcs[:, 0:1],
            )
            nc.sync.dma_start(out.ap(), res[:])

    nc.compile()
    return nc


def kernel(x, conv_w, conv_b, q_w, q_b, k_w, k_b):
    global _k1, _k2
    x = np.asarray(x, dtype=np.float32)
    conv_w = np.asarray(conv_w, dtype=np.float32)
    conv_b = np.asarray(conv_b, dtype=np.float32)
    q_w = np.asarray(q_w, dtype=np.float32)
    k_w = np.asarray(k_w, dtype=np.float32)
    k_b = np.asarray(k_b, dtype=np.float32)

    xf = x.reshape(B, HW, C)
    x_bf = xf.astype(ml_dtypes.bfloat16)
    core_ids = list(range(N_CORES))

    if _k1 is None:
        _k1 = _build_k1()
    in1 = [{"xb": np.ascontiguousarray(x_bf[b])} for b in range(B)]
    r1_res = _run_spmd(_k1, in1, core_ids)
    last_results["k1"] = r1_res
    # raw (128, 4) channel-chunked colsums, one per batch
    sraw = [
        np.asarray(r1_res.results[b]["xsum"], dtype=np.float32).reshape(P, NCH)
        for b in range(B)
    ]

    if _k2 is None:
        _k2 = _build_k2()
    eye = np.eye(C, dtype=np.float32)
    r1_np = np.ascontiguousarray(conv_w.T + eye).astype(ml_dtypes.bfloat16)
    r2_np = np.ascontiguousarray(conv_w + eye).astype(ml_dtypes.bfloat16)
    kwt_np = np.ascontiguousarray(k_w.T * np.float32(SCALE0)).astype(ml_dtypes.bfloat16)
    qw_np = np.ascontiguousarray(q_w).astype(ml_dtypes.bfloat16)
    cbt_np = np.ascontiguousarray(np.stack([
        (conv_b * np.float32(BIAS_MULT)).reshape(NCH, P),
        (k_b * np.float32(ATT_SCALE)).reshape(NCH, P),
    ])[None])  # (1, 2, 4, 128)
    xT_np = np.ascontiguousarray(np.transpose(x_bf, (0, 2, 1)))  # (B, C, HW)
    in2 = []
    for b in range(B):
        others = np.stack([sraw[bb] for bb in range(B) if bb != b])  # (7, 128, 4)
        xso_np = np.ascontiguousarray(np.transpose(others, (0, 2, 1)))  # (7, 4, 128)
        in2.append(
            {
                "xso": xso_np,
                "cbt": cbt_np,
                "r1": r1_np,
                "k_wt": kwt_np,
                "q_w": qw_np,
                "r2": r2_np,
                "xT": xT_np[b],
            }
        )
    r2_res = _run_spmd(_k2, in2, core_ids)
    last_results["k2"] = r2_res
    # out tile[p, t] = result pixel t*128+p  ->  (HW,) per batch
    outs = []
    for b in range(B):
        arr = np.asarray(r2_res.results[b]["out"], dtype=np.float32).reshape(P, NT)
        outs.append(arr.T.reshape(HW))
    return np.stack(outs).astype(np.float32)
